# revision 14
# baseline (speedup 1.0000x reference)
"""Asymmetric L1 loss on 8 Trainium2 NeuronCores.

reference: loss = sum(where(d<0, -penalty[j]*d, d)) / N  with d = computed - target.

Identity used: where(d<0, -p*d, d) = a_j*d + b_j*|d| with a=(1-p)/2, b=(1+p)/2.
So each core only needs per-column partial sums  B_j = sum(d),  A_j = sum(|d|)
over its shard; the penalty weighting + final scalar reduction happen on host
during the gather/unshard step (512 floats total).

Device kernel per core (data-parallel over rows; v6):
  - HWDGE f32 DMA, 2 MiB per transfer ([128, 4096] f32 tiles) on the sync
    queue: steady-state 427-430 GB/s measured (98-99% of the 435 GB/s
    per-core SBUF-AXI fabric ceiling; dual-ring and 4 MiB both measured
    slower).  Landing tiles are freed right after the subtract so DMA never
    waits on the long matmul chain.
  - DVE:  d = c - t into a separate bf16 tile (output cast is free; bf16
    keeps PE in 1-pass matmuls — fp32 matmuls are HI/LO 2-pass and were the
    95%-busy bottleneck in v1)
  - ACT:  a = |d| bf16
  - PE :  ones[128,1].T @ d-chunk  -> psum_B[1,512]  (accumulating matmuls)
          ones[128,1].T @ a-chunk  -> psum_A[1,512]
    (free index f of psum maps to column j = f % 32 — preserved because all
     chunk offsets are multiples of 32; PSUM accumulates fp32 so only the
     bf16 quantization of d enters the error, ~1e-5 on the final loss)
  - tail (v6): the last two 4096-tiles are graded down to
    [2048,2048,1024,1024,1024,512,256,128,128]; the final 4 pieces
    accumulate into a separate [1,128] PSUM pair via 128-wide matmuls, land
    in DEDICATED SBUF pools (so every tail DMA issues with no buffer-reuse
    dep and the queue never starves late), and the main [1,512] accumulators
    are reduced + DMA'd out under the tail's DMA shadow.  Post-last-packet
    drain is ~3.9 us (sem latency + sub + abs + mm + reduce + out-DMA),
    the structural floor for this pipeline.
  - epilogue: strided tensor_reduce folds each accumulator to [1,32]; two
    out-DMAs write the halves of out[1,128] = [B|A|B_tail|A_tail]; penalty
    weighting + cross-core summing happen on host.

Perf note: exec time is bimodal (~337 us clean vs ~410 us contended).  All
8 cores are the 8 NCs of ONE Trn2 device; an HBM stack (~758 GB/s) is
shared by an NC pair and 8 x 429 GB/s oversubscribes the chip by ~10%, so
1-3 randomly-arbitrated victim cores drop to ~330 GB/s while their partner
keeps full rate.  Pacing all cores to the fair share (~379 GB/s -> ~372 us
deterministic) is worse in expectation than this lottery (victim odds ~25%
for the profiled core), so the kernel keeps max rate.
"""

import os
import sys

import numpy as np

if "/opt/trn_rl_repo" not in sys.path:
    sys.path.insert(0, "/opt/trn_rl_repo")

N_TOTAL = 4194304
M_COLS = 32
N_CORES = 8
N_PER_CORE = N_TOTAL // N_CORES          # 524288 rows per core
FLAT = N_PER_CORE * M_COLS               # 16777216 f32 per tensor per core
P = 128
PER_PART = FLAT // P                     # 131072 f32 per partition
TILE_F = 4096                            # f32 elems per partition per tile (2 MiB DMA)
MM_F = 512                               # matmul chunk (PSUM bank = 512 f32)

_cache = {}
_last_results = None


TAIL2_SIZES = [2048, 2048, 1024, 1024, 1024, 512, 256, 128, 128]
TAIL2_NPIECES = 4                 # last 4 pieces (512+256+128+128 cols) -> tail psums


def schedule(n_tiles=PER_PART // TILE_F, tile_f=TILE_F, graded_tail=True, dma_f=None,
             tail2=False):
    if tail2:
        assert sum(TAIL2_SIZES) == 2 * tile_f
        pieces = []
        off = 0
        for _ in range(n_tiles - 2):
            pieces.append((off, tile_f))
            off += tile_f
        for sz in TAIL2_SIZES:
            pieces.append((off, sz))
            off += sz
        return pieces, off
    if dma_f and dma_f != tile_f:
        # large-DMA schedule: dense dma_f-sized transfers, graded tail
        per_part = n_tiles * tile_f
        assert dma_f == 8192 and per_part % dma_f == 0
        tail_sizes = [4096, 4096, 2048, 2048, 1024, 1024, 512, 512, 512, 512]
        assert sum(tail_sizes) == 2 * dma_f
        pieces = []
        off = 0
        for _ in range(per_part // dma_f - 2):
            pieces.append((off, dma_f))
            off += dma_f
        for sz in tail_sizes:
            pieces.append((off, sz))
            off += sz
        return pieces, off
    pieces = []
    off = 0
    if graded_tail and n_tiles > 2:
        tail_sizes = [2048, 2048, 1024, 1024, 1024, 512, 512]
        assert sum(tail_sizes) == 2 * tile_f
        for _ in range(n_tiles - 2):
            pieces.append((off, tile_f))
            off += tile_f
        for sz in tail_sizes:
            pieces.append((off, sz))
            off += sz
    else:
        for _ in range(n_tiles):
            pieces.append((off, tile_f))
            off += tile_f
    return pieces, off


def host_pretile(shard_2d, pieces):
    """Reorder a [P, per_part] shard so each piece's [P, sz] tile is one
    contiguous DRAM block (partition-major within the block)."""
    blocks = [np.ascontiguousarray(shard_2d[:, off:off + sz]).reshape(-1)
              for off, sz in pieces]
    return np.concatenate(blocks)


def build(n_tiles=PER_PART // TILE_F, per_part=PER_PART, tile_f=TILE_F,
          graded_tail=True, warmup=False, dual_ring=False, contig=False,
          dma_f=None, tail2=False):
    from concourse import bacc, mybir, tile

    assert n_tiles * tile_f == per_part
    # schedule: big tiles, with the final two tiles graded down to small
    # pieces so the post-last-DMA chain (sub -> abs -> matmuls -> epilogue)
    # drains fast
    pieces, off = schedule(n_tiles, tile_f, graded_tail, dma_f=dma_f, tail2=tail2)
    assert off == per_part
    land_f = max(sz for _, sz in pieces)     # landing-tile width (DMA size)

    nc = bacc.Bacc(None, target_bir_lowering=False)
    f32 = mybir.dt.float32
    bf16 = mybir.dt.bfloat16
    if contig:
        # pre-tiled layout: each piece's [P, sz] tile is one dense DRAM block
        c_dram = nc.declare_dram_parameter("computed", [P * per_part], f32, isOutput=False)
        t_dram = nc.declare_dram_parameter("target", [P * per_part], f32, isOutput=False)

        def src(dram, off, sz):
            base = off * P
            return dram[base:base + P * sz].rearrange("(p f) -> p f", p=P)
    else:
        c_dram = nc.declare_dram_parameter("computed", [P, per_part], f32, isOutput=False)
        t_dram = nc.declare_dram_parameter("target", [P, per_part], f32, isOutput=False)

        def src(dram, off, sz):
            return dram[:, off:off + sz]
    out_cols = 128 if tail2 else 64
    out_dram = nc.declare_dram_parameter("out", [1, out_cols], f32, isOutput=True)

    cbufs = 4 if land_f <= 4096 else 2       # SBUF: keep pools under ~176KB/partition
    dbufs = 2 if land_f <= 4096 else 3
    if tail2:
        cbufs = 3                            # make room for the dedicated tail pools
    with tile.TileContext(nc) as tc:
        with (
            tc.tile_pool(name="cpool", bufs=cbufs) as cpool,
            tc.tile_pool(name="tpool", bufs=cbufs) as tpool,
            tc.tile_pool(name="dpool", bufs=dbufs) as dpool,
            tc.tile_pool(name="apool", bufs=dbufs) as apool,
            tc.tile_pool(name="ctail", bufs=TAIL2_NPIECES) as ctail_pool,
            tc.tile_pool(name="ttail", bufs=TAIL2_NPIECES) as ttail_pool,
            tc.tile_pool(name="dtail", bufs=TAIL2_NPIECES) as dtail_pool,
            tc.tile_pool(name="atail", bufs=TAIL2_NPIECES) as atail_pool,
            tc.tile_pool(name="kpool", bufs=1) as kpool,
            tc.tile_pool(name="fpool", bufs=1) as fpool,
            tc.tile_pool(name="psum", bufs=1, space="PSUM") as psum_pool,
        ):
            ones = kpool.tile([P, 1], bf16)
            nc.vector.memset(ones[:], 1.0)
            if warmup:
                # tiny DMA to wake the HWDGE ring(s)/SDMA engines before the
                # first real transfer
                wu = kpool.tile([1, 16], f32)
                nc.sync.dma_start(out=wu[:], in_=src(c_dram, 0, 16)[0:1, :])
                if dual_ring:
                    wu2 = kpool.tile([1, 16], f32)
                    nc.scalar.dma_start(out=wu2[:], in_=src(t_dram, 0, 16)[0:1, :])
            psum_b_t = psum_pool.tile([1, MM_F], f32, tag="pb", name="psum_b")
            psum_a_t = psum_pool.tile([1, MM_F], f32, tag="pa", name="psum_a")
            psum_b = psum_b_t[:, :]
            psum_a = psum_a_t[:, :]
            MM2 = 128
            if tail2:
                # separate accumulators for the last TAIL2_NPIECES pieces so
                # the main reduces + out-DMA run under the tail's DMA shadow
                psum_b2_t = psum_pool.tile([1, MM2], f32, tag="pb2", name="psum_b2")
                psum_a2_t = psum_pool.tile([1, MM2], f32, tag="pa2", name="psum_a2")
                psum_b2 = psum_b2_t[:, :]
                psum_a2 = psum_a2_t[:, :]
                tail_first = len(pieces) - TAIL2_NPIECES
            else:
                tail_first = len(pieces)

            def emit_main_epilogue():
                res = fpool.tile([1, 64], f32, tag="rm")
                nc.vector.tensor_reduce(
                    out=res[:, 0:32],
                    in_=psum_b.rearrange("p (r j) -> p j r", j=M_COLS),
                    axis=mybir.AxisListType.X,
                    op=mybir.AluOpType.add,
                )
                nc.vector.tensor_reduce(
                    out=res[:, 32:64],
                    in_=psum_a.rearrange("p (r j) -> p j r", j=M_COLS),
                    axis=mybir.AxisListType.X,
                    op=mybir.AluOpType.add,
                )
                nc.sync.dma_start(out=out_dram[:, 0:64], in_=res[:])

            tail_land_f = max(TAIL2_SIZES[-TAIL2_NPIECES:]) if tail2 else 0
            for pi, (off, sz) in enumerate(pieces):
                in_tail = pi >= tail_first
                if in_tail:
                    # dedicated pools: every tail DMA issues with no
                    # buffer-reuse dep, so the queue never starves late
                    c = ctail_pool.tile([P, tail_land_f], f32, tag="c2")
                    t = ttail_pool.tile([P, tail_land_f], f32, tag="t2")
                else:
                    c = cpool.tile([P, land_f], f32, tag="c")
                    t = tpool.tile([P, land_f], f32, tag="t")
                nc.sync.dma_start(out=c[:, 0:sz], in_=src(c_dram, off, sz))
                t_dma = nc.scalar if dual_ring else nc.sync
                t_dma.dma_start(out=t[:, 0:sz], in_=src(t_dram, off, sz))
                last_piece = pi == len(pieces) - 1
                for so in range(0, sz, tile_f):
                    cs = min(tile_f, sz - so)
                    if in_tail:
                        d = dtail_pool.tile([P, tail_land_f], bf16, tag="d2")
                        a = atail_pool.tile([P, tail_land_f], bf16, tag="a2")
                    else:
                        d = dpool.tile([P, tile_f], bf16, tag="d")
                        a = apool.tile([P, tile_f], bf16, tag="a")
                    nc.vector.tensor_sub(
                        out=d[:, 0:cs], in0=c[:, so:so + cs], in1=t[:, so:so + cs]
                    )
                    nc.scalar.activation(
                        out=a[:, 0:cs], in_=d[:, 0:cs],
                        func=mybir.ActivationFunctionType.Abs,
                    )
                    mm_f = MM2 if in_tail else MM_F
                    pb = psum_b2 if in_tail else psum_b
                    pa = psum_a2 if in_tail else psum_a
                    for m in range(cs // mm_f):
                        if in_tail:
                            first = pi == tail_first and so == 0 and m == 0
                            last = (
                                last_piece and so + tile_f >= sz
                                and m == cs // mm_f - 1
                            )
                        else:
                            first = pi == 0 and so == 0 and m == 0
                            last = (
                                pi == tail_first - 1 and so + tile_f >= sz
                                and m == cs // mm_f - 1
                            )
                        nc.tensor.matmul(
                            pb, ones[:], d[:, m * mm_f:(m + 1) * mm_f],
                            start=first, stop=last,
                        )
                        nc.tensor.matmul(
                            pa, ones[:], a[:, m * mm_f:(m + 1) * mm_f],
                            start=first, stop=last,
                        )
                if tail2 and pi == tail_first - 1:
                    emit_main_epilogue()

            if tail2:
                res2 = fpool.tile([1, 64], f32, tag="rt")
                nc.vector.tensor_reduce(
                    out=res2[:, 0:32],
                    in_=psum_b2.rearrange("p (r j) -> p j r", j=M_COLS),
                    axis=mybir.AxisListType.X,
                    op=mybir.AluOpType.add,
                )
                nc.vector.tensor_reduce(
                    out=res2[:, 32:64],
                    in_=psum_a2.rearrange("p (r j) -> p j r", j=M_COLS),
                    axis=mybir.AxisListType.X,
                    op=mybir.AluOpType.add,
                )
                nc.sync.dma_start(out=out_dram[:, 64:128], in_=res2[:])
            else:
                emit_main_epilogue()

    nc.compile()
    return nc


DEFAULT_OPTS = {"contig": True, "tail2": True}


def kernel(computed, target, penalty):
    global _last_results
    from concourse.bass_utils import run_bass_kernel_spmd

    if "nc" not in _cache:
        opts = {**DEFAULT_OPTS, **_cache.get("build_opts", {})}
        tile_f = opts.get("tile_f", TILE_F)
        opts.setdefault("tile_f", tile_f)
        opts.setdefault("n_tiles", PER_PART // tile_f)
        _cache["opts"] = opts
        _cache["nc"] = build(**opts)
    nc = _cache["nc"]
    o = _cache["opts"]
    pieces, _ = schedule(
        n_tiles=o["n_tiles"],
        tile_f=o["tile_f"],
        graded_tail=o.get("graded_tail", True),
        dma_f=o.get("dma_f"),
        tail2=o.get("tail2", False),
    )

    computed = np.ascontiguousarray(computed, dtype=np.float32)
    target = np.ascontiguousarray(target, dtype=np.float32)
    in_maps = []
    for i in range(N_CORES):
        sl = slice(i * N_PER_CORE, (i + 1) * N_PER_CORE)
        in_maps.append(
            {
                "computed": host_pretile(computed[sl].reshape(P, PER_PART), pieces),
                "target": host_pretile(target[sl].reshape(P, PER_PART), pieces),
            }
        )

    trace = bool(os.environ.get("KERNEL_TRACE"))
    res = run_bass_kernel_spmd(nc, in_maps, core_ids=list(range(N_CORES)), trace=trace)
    _last_results = res

    B = np.zeros(M_COLS, np.float64)
    A = np.zeros(M_COLS, np.float64)
    tail2 = o.get("tail2", False)
    for r in res.results:
        out = np.asarray(r["out"]).reshape(-1).astype(np.float64)
        B += out[:32]
        A += out[32:64]
        if tail2:
            B += out[64:96]
            A += out[96:128]
    p = np.asarray(penalty, dtype=np.float64)
    aw = (1.0 - p) / 2.0
    bw = (1.0 + p) / 2.0
    total = float(aw @ B + bw @ A)
    return np.float32(total / N_TOTAL)



# revision 35
# speedup vs baseline: 1.2495x; 1.2495x over previous
"""Asymmetric L1 loss on 8 Trainium2 NeuronCores.

reference: loss = sum(where(d<0, -penalty[j]*d, d)) / N  with d = computed - target.

Identity used: where(d<0, -p*d, d) = a_j*d + b_j*|d| with a=(1-p)/2, b=(1+p)/2.
So each core only needs per-column partial sums  B_j = sum(d),  A_j = sum(|d|)
over its shard; the penalty weighting + final scalar reduction happen on host
during the gather/unshard step (512 floats total).

Device kernel per core (data-parallel over rows; v7):
  - INTERLEAVED single-stream DMA (v7): host pretiles computed|target into
    one DRAM buffer where each piece is a dense [128, 2*sz] block (c | t).
    One HWDGE DMA per piece on the sync queue moves both tensors with
    32 KiB per-partition packets (2x v6), halving per-packet overhead and
    DMA instruction count: steady-state 421-442 GB/s measured vs 427-430
    for split streams (ceiling 435).  8 MiB pieces (64 KiB packets)
    measured WORSE (+6.7us overhead, trips the util throttle harder), as
    did dual-ring and non-interleaved 4 MiB.  Landing tiles are freed
    right after the subtract so DMA never waits on the matmul chain.
  - DVE:  d = c - t into a separate bf16 tile (output cast is free; bf16
    keeps PE in 1-pass matmuls — fp32 matmuls are HI/LO 2-pass and were the
    95%-busy bottleneck in v1)
  - ACT:  a = |d| bf16
  - PE :  ones[128,1].T @ d-chunk  -> psum_B[1,512]  (accumulating matmuls)
          ones[128,1].T @ a-chunk  -> psum_A[1,512]
    (free index f of psum maps to column j = f % 32 — preserved because all
     chunk offsets are multiples of 32; PSUM accumulates fp32 so only the
     bf16 quantization of d enters the error, ~1e-5 on the final loss)
  - tail (v6): the last two 4096-tiles are graded down to
    [2048,2048,1024,1024,1024,512,256,128,128]; the final 4 pieces
    accumulate into a separate [1,128] PSUM pair via 128-wide matmuls, land
    in DEDICATED SBUF pools (so every tail DMA issues with no buffer-reuse
    dep and the queue never starves late), and the main [1,512] accumulators
    are reduced + DMA'd out under the tail's DMA shadow.  Post-last-packet
    drain is ~3.9 us (sem latency + sub + abs + mm + reduce + out-DMA),
    the structural floor for this pipeline.
  - epilogue: strided tensor_reduce folds each accumulator to [1,32]; two
    out-DMAs write the halves of out[1,128] = [B|A|B_tail|A_tail]; penalty
    weighting + cross-core summing happen on host.

Perf note: exec time is bimodal (~337 us clean vs ~410 us contended).  All
8 cores are the 8 NCs of ONE Trn2 device; an HBM stack (~758 GB/s) is
shared by an NC pair and 8 x 429 GB/s oversubscribes the chip by ~10%, so
1-3 randomly-arbitrated victim cores drop to ~330 GB/s while their partner
keeps full rate.  Pacing all cores to the fair share (~379 GB/s -> ~372 us
deterministic) is worse in expectation than this lottery (victim odds ~25%
for the profiled core), so the kernel keeps max rate.
"""

import os
import sys

import numpy as np

if "/opt/trn_rl_repo" not in sys.path:
    sys.path.insert(0, "/opt/trn_rl_repo")

N_TOTAL = 4194304
M_COLS = 32
N_CORES = 8
N_PER_CORE = N_TOTAL // N_CORES          # 524288 rows per core
FLAT = N_PER_CORE * M_COLS               # 16777216 f32 per tensor per core
P = 128
PER_PART = FLAT // P                     # 131072 f32 per partition
TILE_F = 4096                            # f32 elems per partition per tile (2 MiB DMA)
MM_F = 512                               # matmul chunk (PSUM bank = 512 f32)

_cache = {}
_last_results = None


TAIL2_SIZES = [2048, 2048, 1024, 1024, 1024, 512, 256, 128, 128]
TAIL2_NPIECES = 4                 # last 4 pieces (512+256+128+128 cols) -> tail psums.
                                  # A bigger tail group measures WORSE: 128-wide tail
                                  # matmul pairs cost ~300ns each on PE, so >8 pairs
                                  # exceed the tail's DMA shadow and delay the stop.


def schedule(n_tiles=PER_PART // TILE_F, tile_f=TILE_F, graded_tail=True, dma_f=None,
             tail2=False, dma8=False):
    if tail2:
        assert sum(TAIL2_SIZES) == 2 * tile_f
        pieces = []
        off = 0
        if dma8:
            # 8192-wide main DMA pieces (compute still chunks at tile_f)
            for _ in range((n_tiles - 2) // 2):
                pieces.append((off, 2 * tile_f))
                off += 2 * tile_f
        else:
            for _ in range(n_tiles - 2):
                pieces.append((off, tile_f))
                off += tile_f
        for sz in TAIL2_SIZES:
            pieces.append((off, sz))
            off += sz
        return pieces, off
    if dma_f and dma_f != tile_f:
        # large-DMA schedule: dense dma_f-sized transfers, graded tail
        per_part = n_tiles * tile_f
        assert dma_f == 8192 and per_part % dma_f == 0
        tail_sizes = [4096, 4096, 2048, 2048, 1024, 1024, 512, 512, 512, 512]
        assert sum(tail_sizes) == 2 * dma_f
        pieces = []
        off = 0
        for _ in range(per_part // dma_f - 2):
            pieces.append((off, dma_f))
            off += dma_f
        for sz in tail_sizes:
            pieces.append((off, sz))
            off += sz
        return pieces, off
    pieces = []
    off = 0
    if graded_tail and n_tiles > 2:
        tail_sizes = [2048, 2048, 1024, 1024, 1024, 512, 512]
        assert sum(tail_sizes) == 2 * tile_f
        for _ in range(n_tiles - 2):
            pieces.append((off, tile_f))
            off += tile_f
        for sz in tail_sizes:
            pieces.append((off, sz))
            off += sz
    else:
        for _ in range(n_tiles):
            pieces.append((off, tile_f))
            off += tile_f
    return pieces, off


def host_pretile(shard_2d, pieces):
    """Reorder a [P, per_part] shard so each piece's [P, sz] tile is one
    contiguous DRAM block (partition-major within the block)."""
    blocks = [np.ascontiguousarray(shard_2d[:, off:off + sz]).reshape(-1)
              for off, sz in pieces]
    return np.concatenate(blocks)


def host_pretile_ilv(c_2d, t_2d, pieces):
    """Interleave computed/target per piece: each piece becomes one dense
    [P, 2*sz] DRAM block ([:, 0:sz]=c, [:, sz:2sz]=t) so a single DMA per
    piece moves both tensors with 2x-size per-partition packets."""
    blocks = [
        np.ascontiguousarray(
            np.concatenate([c_2d[:, off:off + sz], t_2d[:, off:off + sz]], axis=1)
        ).reshape(-1)
        for off, sz in pieces
    ]
    return np.concatenate(blocks)


def build(n_tiles=PER_PART // TILE_F, per_part=PER_PART, tile_f=TILE_F,
          graded_tail=True, warmup=False, dual_ring=False, contig=False,
          dma_f=None, tail2=False, ilv=False, dma8=False, cbufs=None):
    from concourse import bacc, mybir, tile

    assert n_tiles * tile_f == per_part
    # schedule: big tiles, with the final two tiles graded down to small
    # pieces so the post-last-DMA chain (sub -> abs -> matmuls -> epilogue)
    # drains fast
    pieces, off = schedule(n_tiles, tile_f, graded_tail, dma_f=dma_f, tail2=tail2,
                           dma8=dma8)
    assert off == per_part
    land_f = max(sz for _, sz in pieces)     # landing-tile width (DMA size)

    nc = bacc.Bacc(None, target_bir_lowering=False)
    f32 = mybir.dt.float32
    bf16 = mybir.dt.bfloat16
    if ilv:
        # interleaved layout: each piece is ONE dense [P, 2*sz] block
        # (c | t) -> one DMA per piece, 2x per-partition packet size
        data_dram = nc.declare_dram_parameter(
            "data", [P * 2 * per_part], f32, isOutput=False
        )

        def src_ilv(off, sz):
            base = 2 * off * P
            return data_dram[base:base + P * 2 * sz].rearrange("(p f) -> p f", p=P)

        c_dram = t_dram = None
        src = None
    elif contig:
        # pre-tiled layout: each piece's [P, sz] tile is one dense DRAM block
        c_dram = nc.declare_dram_parameter("computed", [P * per_part], f32, isOutput=False)
        t_dram = nc.declare_dram_parameter("target", [P * per_part], f32, isOutput=False)

        def src(dram, off, sz):
            base = off * P
            return dram[base:base + P * sz].rearrange("(p f) -> p f", p=P)
    else:
        c_dram = nc.declare_dram_parameter("computed", [P, per_part], f32, isOutput=False)
        t_dram = nc.declare_dram_parameter("target", [P, per_part], f32, isOutput=False)

        def src(dram, off, sz):
            return dram[:, off:off + sz]
    out_cols = 128 if tail2 else 64
    out_dram = nc.declare_dram_parameter("out", [1, out_cols], f32, isOutput=True)

    if cbufs is None:
        cbufs = 4 if land_f <= 4096 else 2   # SBUF: keep pools under ~176KB/partition
        if tail2:
            cbufs = 3                        # make room for the dedicated tail pools
        if ilv and land_f > tile_f:
            cbufs = 2                        # [P, 2*8192] f32 tiles are 64KB/partition
    dbufs = 2 if land_f <= 4096 or dma8 else 3
    with tile.TileContext(nc) as tc:
        with (
            tc.tile_pool(name="cpool", bufs=cbufs) as cpool,
            tc.tile_pool(name="tpool", bufs=cbufs) as tpool,
            tc.tile_pool(name="dpool", bufs=dbufs) as dpool,
            tc.tile_pool(name="apool", bufs=dbufs) as apool,
            tc.tile_pool(name="ctail", bufs=TAIL2_NPIECES) as ctail_pool,
            tc.tile_pool(name="ttail", bufs=TAIL2_NPIECES) as ttail_pool,
            tc.tile_pool(name="dtail", bufs=TAIL2_NPIECES) as dtail_pool,
            tc.tile_pool(name="atail", bufs=TAIL2_NPIECES) as atail_pool,
            tc.tile_pool(name="kpool", bufs=1) as kpool,
            tc.tile_pool(name="fpool", bufs=1) as fpool,
            tc.tile_pool(name="psum", bufs=1, space="PSUM") as psum_pool,
        ):
            ones = kpool.tile([P, 1], bf16)
            nc.vector.memset(ones[:], 1.0)
            if warmup:
                # tiny DMA to wake the HWDGE ring(s)/SDMA engines before the
                # first real transfer
                wu = kpool.tile([1, 16], f32)
                nc.sync.dma_start(out=wu[:], in_=src(c_dram, 0, 16)[0:1, :])
                if dual_ring:
                    wu2 = kpool.tile([1, 16], f32)
                    nc.scalar.dma_start(out=wu2[:], in_=src(t_dram, 0, 16)[0:1, :])
            psum_b_t = psum_pool.tile([1, MM_F], f32, tag="pb", name="psum_b")
            psum_a_t = psum_pool.tile([1, MM_F], f32, tag="pa", name="psum_a")
            psum_b = psum_b_t[:, :]
            psum_a = psum_a_t[:, :]
            MM2 = 128
            if tail2:
                # separate accumulators for the last TAIL2_NPIECES pieces so
                # the main reduces + out-DMA run under the tail's DMA shadow
                psum_b2_t = psum_pool.tile([1, MM2], f32, tag="pb2", name="psum_b2")
                psum_a2_t = psum_pool.tile([1, MM2], f32, tag="pa2", name="psum_a2")
                psum_b2 = psum_b2_t[:, :]
                psum_a2 = psum_a2_t[:, :]
                tail_first = len(pieces) - TAIL2_NPIECES
            else:
                tail_first = len(pieces)

            def emit_main_epilogue():
                res = fpool.tile([1, 64], f32, tag="rm")
                nc.vector.tensor_reduce(
                    out=res[:, 0:32],
                    in_=psum_b.rearrange("p (r j) -> p j r", j=M_COLS),
                    axis=mybir.AxisListType.X,
                    op=mybir.AluOpType.add,
                )
                nc.vector.tensor_reduce(
                    out=res[:, 32:64],
                    in_=psum_a.rearrange("p (r j) -> p j r", j=M_COLS),
                    axis=mybir.AxisListType.X,
                    op=mybir.AluOpType.add,
                )
                nc.sync.dma_start(out=out_dram[:, 0:64], in_=res[:])

            tail_land_f = max(TAIL2_SIZES[-TAIL2_NPIECES:]) if tail2 else 0
            for pi, (off, sz) in enumerate(pieces):
                in_tail = pi >= tail_first
                if ilv:
                    wid = tail_land_f if in_tail else land_f
                    pool = ctail_pool if in_tail else cpool
                    ct = pool.tile([P, 2 * wid], f32, tag="ct2" if in_tail else "ct")
                    nc.sync.dma_start(out=ct[:, 0:2 * sz], in_=src_ilv(off, sz))
                    c = ct[:, 0:sz]
                    t = ct[:, sz:2 * sz]
                else:
                    if in_tail:
                        # dedicated pools: every tail DMA issues with no
                        # buffer-reuse dep, so the queue never starves late
                        c = ctail_pool.tile([P, tail_land_f], f32, tag="c2")
                        t = ttail_pool.tile([P, tail_land_f], f32, tag="t2")
                    else:
                        c = cpool.tile([P, land_f], f32, tag="c")
                        t = tpool.tile([P, land_f], f32, tag="t")
                    nc.sync.dma_start(out=c[:, 0:sz], in_=src(c_dram, off, sz))
                    t_dma = nc.scalar if dual_ring else nc.sync
                    t_dma.dma_start(out=t[:, 0:sz], in_=src(t_dram, off, sz))
                last_piece = pi == len(pieces) - 1
                for so in range(0, sz, tile_f):
                    cs = min(tile_f, sz - so)
                    if in_tail:
                        d = dtail_pool.tile([P, tail_land_f], bf16, tag="d2")
                        a = atail_pool.tile([P, tail_land_f], bf16, tag="a2")
                    else:
                        d = dpool.tile([P, tile_f], bf16, tag="d")
                        a = apool.tile([P, tile_f], bf16, tag="a")
                    # tail SUBs go to the idle Pool engine so they don't
                    # queue behind the main PSUM reduces on the in-order DVE
                    sub_eng = nc.gpsimd if in_tail else nc.vector
                    sub_eng.tensor_sub(
                        out=d[:, 0:cs], in0=c[:, so:so + cs], in1=t[:, so:so + cs]
                    )
                    nc.scalar.activation(
                        out=a[:, 0:cs], in_=d[:, 0:cs],
                        func=mybir.ActivationFunctionType.Abs,
                    )
                    mm_f = MM2 if in_tail else MM_F
                    pb = psum_b2 if in_tail else psum_b
                    pa = psum_a2 if in_tail else psum_a
                    for m in range(cs // mm_f):
                        if in_tail:
                            first = pi == tail_first and so == 0 and m == 0
                            last = (
                                last_piece and so + tile_f >= sz
                                and m == cs // mm_f - 1
                            )
                        else:
                            first = pi == 0 and so == 0 and m == 0
                            last = (
                                pi == tail_first - 1 and so + tile_f >= sz
                                and m == cs // mm_f - 1
                            )
                        nc.tensor.matmul(
                            pb, ones[:], d[:, m * mm_f:(m + 1) * mm_f],
                            start=first, stop=last,
                        )
                        nc.tensor.matmul(
                            pa, ones[:], a[:, m * mm_f:(m + 1) * mm_f],
                            start=first, stop=last,
                        )
                if tail2 and pi == tail_first - 1:
                    emit_main_epilogue()

            if tail2:
                res2 = fpool.tile([1, 64], f32, tag="rt")
                nc.vector.tensor_reduce(
                    out=res2[:, 0:32],
                    in_=psum_b2.rearrange("p (r j) -> p j r", j=M_COLS),
                    axis=mybir.AxisListType.X,
                    op=mybir.AluOpType.add,
                )
                nc.vector.tensor_reduce(
                    out=res2[:, 32:64],
                    in_=psum_a2.rearrange("p (r j) -> p j r", j=M_COLS),
                    axis=mybir.AxisListType.X,
                    op=mybir.AluOpType.add,
                )
                nc.sync.dma_start(out=out_dram[:, 64:128], in_=res2[:])
            else:
                emit_main_epilogue()

    nc.compile()
    return nc


DEFAULT_OPTS = {"contig": True, "tail2": True, "ilv": True}


def kernel(computed, target, penalty):
    global _last_results
    from concourse.bass_utils import run_bass_kernel_spmd

    if "nc" not in _cache:
        opts = {**DEFAULT_OPTS, **_cache.get("build_opts", {})}
        tile_f = opts.get("tile_f", TILE_F)
        opts.setdefault("tile_f", tile_f)
        opts.setdefault("n_tiles", PER_PART // tile_f)
        _cache["opts"] = opts
        _cache["nc"] = build(**opts)
    nc = _cache["nc"]
    o = _cache["opts"]
    pieces, _ = schedule(
        n_tiles=o["n_tiles"],
        tile_f=o["tile_f"],
        graded_tail=o.get("graded_tail", True),
        dma_f=o.get("dma_f"),
        tail2=o.get("tail2", False),
        dma8=o.get("dma8", False),
    )

    computed = np.ascontiguousarray(computed, dtype=np.float32)
    target = np.ascontiguousarray(target, dtype=np.float32)
    in_maps = []
    for i in range(N_CORES):
        sl = slice(i * N_PER_CORE, (i + 1) * N_PER_CORE)
        c2d = computed[sl].reshape(P, PER_PART)
        t2d = target[sl].reshape(P, PER_PART)
        if o.get("ilv", False):
            in_maps.append({"data": host_pretile_ilv(c2d, t2d, pieces)})
        else:
            in_maps.append(
                {
                    "computed": host_pretile(c2d, pieces),
                    "target": host_pretile(t2d, pieces),
                }
            )

    trace = bool(os.environ.get("KERNEL_TRACE"))
    res = run_bass_kernel_spmd(nc, in_maps, core_ids=list(range(N_CORES)), trace=trace)
    _last_results = res

    B = np.zeros(M_COLS, np.float64)
    A = np.zeros(M_COLS, np.float64)
    tail2 = o.get("tail2", False)
    for r in res.results:
        out = np.asarray(r["out"]).reshape(-1).astype(np.float64)
        B += out[:32]
        A += out[32:64]
        if tail2:
            B += out[64:96]
            A += out[96:128]
    p = np.asarray(penalty, dtype=np.float64)
    aw = (1.0 - p) / 2.0
    bw = (1.0 + p) / 2.0
    total = float(aw @ B + bw @ A)
    return np.float32(total / N_TOTAL)

